# revision 31
# baseline (speedup 1.0000x reference)
"""Trainium2 Bass kernel for nn_AtenMmQuint8: quint8 dense matmul.

    out = ((x - 65) * 0.199) @ ((y - 160) * 0.0215)
    x: [2048, 4096] int32 (quint8 values 0..255)
    y: [4096, 2048] int32 (quint8 values 0..255)
    out: [2048, 2048] fp32

Sharding: 4x2 tensor-parallel grid over the 8 NeuronCores (4 M-blocks x
2 N-blocks); each core computes a 512x1024 output block with full K.

Math: fp8 DoubleRow matmul (2x PE throughput over bf16) with an exact
rank-1 correction that rides for free in the output pass:

    (x-65)@(y-160) = x~@(y~ - 32) + eps
                   = x~@y~ - 32*rowsum(x~)[m] + eps

where x~ = fp8e4m3(x-65) (so the x-side zero-point offset is zero and
no per-column correction is needed) and y~ = fp8e4m3(y-128) (centered
so |v|<=128 and the y-side quant error is halved; its -32 offset folds
into the per-output-row correction corrA[m] = -32*rowsum(x~)[m],
computed on the host from the *quantized* x). The remaining eps is
zero-mean fp8 cross-term noise: measured max rel err 1.18e-2 on the
actual problem seed (threshold 2e-2). All products and partial sums
are integers < 2^24, so fp32 PSUM accumulation is bit-exact and the
host-side error simulation is a faithful predictor of hardware.

Host staging: x (K-major, transposed) and y are quantized to fp8
bytes and staged CONCATENATED per core ([4096, 512+1024]) so a single
DMA delivers a complete contraction k-range for both operands; corrA
([128, 4] fp32, corrA[p, mg] for output row mg*128+p) rides along as
a tiny side input.

Device kernel (identical SPMD program on all 8 cores):
  - K interleaved across SBUF partitions (k = p*32 + j) so each load
    chunk is 128 large contiguous runs (one per partition); HWDGE
    descriptor generation otherwise serializes the load stream.
  - Progressively-sized load chunks alternate between the SP and ACT
    HWDGE rings (per-ring completion latency is ~2.2us, so two rings
    double the early delivery rate); the first chunk is minimal (one
    k-pair) to unlock the stream as soon as possible. corrA loads
    last on the ACT ring (needed only at output time).
  - PE prewarm (~39 throwaway matmuls, gated only on a GpSimd memset
    -- the first engine queue to boot -- so it starts ~1us in)
    releases the HAM clock gate (1.2 -> 2.4 GHz) during the initial
    load window, which is bounded by the first chunk's DMA completion
    latency (~5-6us).
  - Main stream: 128 DoubleRow fp8 matmuls (16 k-pair-tiles x 4 m x
    2 n), each contracting 2 k-tiles (2x128 virtual K) per pass at
    ~238 ns -- 2x the bf16 matmul roofline. The pair dim is an AP dim
    (stride % 16 == 0), no byte interleave needed. The last
    k-pair-tiles run m-major (and the last m-group n-major) so banks
    retire early and correction+scale+store overlaps the remaining
    matmuls.
  - PSUM -> SBUF as a single fused correction+scale tensor_scalar on
    VectorE: out = (psum + corrA[p]) * SCALE. One store DMA per
    128-row group, alternating HWDGE rings so completions overlap. The
    last bank's matmul and pass are split in half (DVE's semaphore
    pickup after a matmul is ~40ns, so the half gated by the very last
    matmul starts almost immediately), and its store is split by ROWS
    (64x2KB descriptors per ring instead of 128x1KB) so the
    kernel-ending chain (last matmul -> copy -> store -> completion)
    is short.
"""

import numpy as np
import ml_dtypes

import concourse.bass as bass  # noqa: F401  (kept for callers/debugging)
import concourse.mybir as mybir
import concourse.tile as tile
from concourse import bacc
from concourse.bass_utils import run_bass_kernel_spmd

X_ZP, Y_ZP = 65.0, 160.0
SCALE = 0.199 * 0.0215
Y_CENTER = 128.0
YOFF = Y_CENTER - Y_ZP  # -32: y-160 = (y-128) - 32

M, K, N = 2048, 4096, 2048
GM, GN = 4, 2  # core grid: 4 M-blocks x 2 N-blocks
MC, NC = M // GM, N // GN  # 512 x 1024 per-core output block
P = 128  # partitions / k-tile size
NB = 512  # psum bank free size (one fp32 bank; matmul cannot cross banks)
# k-pair-tiles (2 k-tiles each) per load DMA. x and y are staged
# concatenated ([k, mc+nnc]) so one DMA delivers a complete k-pair for
# both operands; chunks alternate between the SP and ACT HWDGE rings,
# doubling the completion rate. Small leading chunks start the
# pipeline, big trailing chunks amortize per-DMA completion latency.
DMA_CHUNKS = (1, 1, 2, 2, 3, 3, 2, 2)
JT_TAIL = 8  # trailing k-pair-tiles run m-major so PSUM banks retire early
# Prewarm fills the PE-idle window until the first chunk's DMA
# completion lands (~5us): covers the HAM ramp exactly.
N_WARM = 39


def _emit(tc, zq, mc, corrA, out, dma_chunks=DMA_CHUNKS,
          jt_tail=JT_TAIL, n_warm=N_WARM):
    """Emit the per-core device program.

    zq: [k, mc+nnc] f8 DRAM ([x~ K-major | y~] concatenated),
    corrA: [P, mt] fp32 DRAM, out: [mc, nnc] fp32 DRAM.
    """
    nc = tc.nc
    k, w = zq.shape
    nnc = w - mc
    kt = k // P
    jt = kt // 2  # DoubleRow k-pair tiles
    mt = mc // P
    nt = nnc // NB
    assert 2 * sum(dma_chunks) == kt

    fp32 = mybir.dt.float32
    bf16 = mybir.dt.bfloat16
    f8 = mybir.dt.float8e4

    with (
        tc.tile_pool(name="sb", bufs=1) as sbp,
        tc.tile_pool(name="osb", bufs=mt, space="SBUF") as osbp,
        tc.tile_pool(name="ps", bufs=mt * nt, space="PSUM") as psp,
    ):
        # Everything is persistent (fits in SBUF at this problem size).
        zs = sbp.tile([P, kt, mc + nnc], f8, name="zs")
        xs = zs[:, :, 0:mc]
        ys = zs[:, :, mc:]
        ca = sbp.tile([P, mt], fp32, name="ca")
        wt = sbp.tile([P, P], bf16, name="wt")
        psum = [
            [psp.tile([P, NB], fp32, tag="ps", name=f"ps_{m}_{n}") for n in range(nt)]
            for m in range(mt)
        ]

        # HAM prewarm: the PE sits idle while the first chunks load;
        # throwaway matmuls release the clock gate to 8/8 before the
        # real stream starts. Gate only on a cheap GpSimd memset --
        # GpSimd's queue is the first to boot, so warmup starts ~0.3us
        # earlier than with a VectorE gate.
        nc.gpsimd.memset(wt[:], 0.0)
        for _ in range(n_warm):
            nc.tensor.matmul(psum[0][0][:, :P], wt[:], wt[:], start=True, stop=True)

        # K is interleaved across partitions (k = p*kt + j): each
        # partition's j-range is one big contiguous DRAM run, so a chunk
        # DMA is 128 descriptors (one per partition). The contraction is
        # a permutation of K, identical for x and y, so the matmul sum
        # is unchanged.
        zqr = zq.rearrange("(p j) m -> p j m", j=kt)
        k0 = 0
        rings = (nc.sync, nc.scalar)
        for i, nj in enumerate(dma_chunks):
            nk = 2 * nj
            rings[i % 2].dma_start(zs[:, k0 : k0 + nk, :], zqr[:, k0 : k0 + nk, :])
            k0 += nk
        # corrA is tiny and only needed by the output pass; load it on
        # the ACT ring after the y chunks so it never delays the stream.
        nc.scalar.dma_start(ca[:], corrA)

        def mm(j2, m, n, nsl=slice(0, NB), psl=slice(0, NB)):
            nc.tensor.matmul(
                psum[m][n][:, psl],
                zs[:, 2 * j2 : 2 * j2 + 2, m * P : (m + 1) * P],
                zs[:, 2 * j2 : 2 * j2 + 2, mc + n * NB + nsl.start : mc + n * NB + nsl.stop],
                start=(j2 == 0),
                stop=(j2 == jt - 1),
                perf_mode=mybir.MatmulPerfMode.DoubleRow,
            )

        # k-outer: touch every psum bank each k-pair-tile so the PE
        # stream stays dense while loads race ahead.
        for j2 in range(jt - jt_tail):
            for m in range(mt):
                for n in range(nt):
                    mm(j2, m, n)
        # m-outer tail: bank group m finishes its K accumulation early so
        # its correction+scale+store overlaps the remaining matmuls. The
        # last m-group runs n-major, and the very last matmul is split
        # in half so the kernel-ending chain is short.
        half = NB // 2
        for m in range(mt):
            if m < mt - 1:
                for j2 in range(jt - jt_tail, jt):
                    for n in range(nt):
                        mm(j2, m, n)
            else:
                for n in range(nt):
                    for j2 in range(jt - jt_tail, jt):
                        if n == nt - 1 and j2 == jt - 1:
                            mm(j2, m, n, nsl=slice(0, half), psl=slice(0, half))
                            mm(j2, m, n, nsl=slice(half, NB), psl=slice(half, NB))
                        else:
                            mm(j2, m, n)

        # Fused correction + scale, PSUM->SBUF: out = (psum + corrA[p])
        # * SCALE on VectorE (ACT helps on the last piece). One store
        # per 128-row group, alternating HWDGE rings so completions
        # overlap; the last group's passes and stores are split so the
        # ending chain (last matmul -> copy -> store) is short.
        def opass(dst, src, m, engine=None):
            (engine or nc.vector).tensor_scalar(
                dst, src, ca[:, m : m + 1], SCALE,
                op0=mybir.AluOpType.add, op1=mybir.AluOpType.mult,
            )

        # Ring/queue assignment keeps the kernel-ending chain unblocked:
        # the ACT engine's strict-FIFO queue must not hold any store
        # whose wait (on a DVE pass) would delay the final activation.
        store_rings = [nc.scalar, nc.sync]
        for m in range(mt):
            osb = osbp.tile([P, nnc], fp32, tag="osb", name=f"osb_{m}")
            ring = store_rings[m % 2]
            last_ring = nc.sync
            if m < mt - 1:
                for n in range(nt):
                    opass(osb[:, n * NB : (n + 1) * NB], psum[m][n][:], m)
                ring.dma_start(out[m * P : (m + 1) * P, :], osb[:])
            else:
                # n=0 bank retires 4 k-pair-tiles early (n-major tail):
                # pass + store while the last matmuls still run.
                opass(osb[:, 0:NB], psum[m][0][:], m)
                last_ring.dma_start(out[m * P : (m + 1) * P, 0:NB], osb[:, 0:NB])
                # n=1 bank: both halves on DVE back-to-back. DVE's
                # semaphore pickup is ~40ns vs ACT's ~610ns (measured
                # consistently), so even serialized DVE halves finish
                # ~200ns sooner than a parallel DVE+ACT split: halfA
                # starts at its matmul (one MM before last), halfB
                # follows immediately, ending ~0.9us after the last
                # matmul.
                opass(osb[:, NB : NB + half], psum[m][1][:, 0:half], m)
                opass(osb[:, NB + half :], psum[m][1][:, half:], m)
                # Store the n=1 block split by ROWS, not columns: each
                # half is 64 descriptors of 2KB instead of 128 of 1KB,
                # halving the per-descriptor completion storm that
                # dominates the kernel-ending DMA; the halves ride
                # different rings in parallel.
                hp = P // 2
                nc.scalar.dma_start(
                    out[m * P : m * P + hp, NB:], osb[:hp, NB:]
                )
                nc.sync.dma_start(
                    out[m * P + hp : (m + 1) * P, NB:], osb[hp:, NB:]
                )


def _build_nc(k=K, mc=MC, nnc=NC, **emit_kw):
    nc = bacc.Bacc("TRN2", target_bir_lowering=False, debug=False)
    zq = nc.declare_dram_parameter(
        "zq", [k, mc + nnc], mybir.dt.float8e4, isOutput=False
    )
    corrA = nc.declare_dram_parameter(
        "corrA", [P, mc // P], mybir.dt.float32, isOutput=False
    )
    out = nc.declare_dram_parameter("out", [mc, nnc], mybir.dt.float32, isOutput=True)
    with tile.TileContext(nc) as tc:
        _emit(tc, zq[:], mc, corrA[:], out[:], **emit_kw)
    nc.compile()
    return nc


_CACHE = {}


def _get_nc():
    if "nc" not in _CACHE:
        _CACHE["nc"] = _build_nc()
    return _CACHE["nc"]


def kernel(x, y):
    x = np.asarray(x)
    y = np.asarray(y)
    assert x.shape == (M, K) and y.shape == (K, N)
    f8 = ml_dtypes.float8_e4m3
    # fp8 quantization (values guaranteed 0..255 by the spec): x keeps
    # its true zero-point (no per-column correction needed), y centered.
    xq = (x.astype(np.float32) - X_ZP).astype(f8)  # [M, K]
    yq = (y.astype(np.float32) - Y_CENTER).astype(f8)  # [K, N]
    # Exact rank-1 correction computed from the *quantized* x.
    Rx = xq.astype(np.float32).sum(axis=1, dtype=np.float64)  # [M]
    corrA_full = (YOFF * Rx).astype(np.float32)  # [M]
    xqT = np.ascontiguousarray(xq.T)  # [K, M] K-major

    in_maps = []
    for i in range(GM * GN):
        mi, ni = divmod(i, GN)
        in_maps.append(
            {
                # x (K-major) and y concatenated so one DMA chunk
                # delivers both operands for a k-range
                "zq": np.ascontiguousarray(
                    np.concatenate(
                        [
                            xqT[:, mi * MC : (mi + 1) * MC],
                            yq[:, ni * NC : (ni + 1) * NC],
                        ],
                        axis=1,
                    )
                ),
                # corrA[p, mg] covers output row mg*128 + p of this block
                "corrA": np.ascontiguousarray(
                    corrA_full[mi * MC : (mi + 1) * MC].reshape(MC // P, P).T
                ),
            }
        )

    res = run_bass_kernel_spmd(_get_nc(), in_maps, list(range(GM * GN)))
    _CACHE["last_results"] = res

    out = np.empty((M, N), np.float32)
    for i in range(GM * GN):
        mi, ni = divmod(i, GN)
        out[mi * MC : (mi + 1) * MC, ni * NC : (ni + 1) * NC] = res.results[i]["out"]
    return out


# revision 32
# speedup vs baseline: 1.0162x; 1.0162x over previous
"""Trainium2 Bass kernel for nn_AtenMmQuint8: quint8 dense matmul.

    out = ((x - 65) * 0.199) @ ((y - 160) * 0.0215)
    x: [2048, 4096] int32 (quint8 values 0..255)
    y: [4096, 2048] int32 (quint8 values 0..255)
    out: [2048, 2048] fp32

Sharding: 4x2 tensor-parallel grid over the 8 NeuronCores (4 M-blocks x
2 N-blocks); each core computes a 512x1024 output block with full K.

Math: fp8 DoubleRow matmul (2x PE throughput over bf16) with an exact
rank-1 correction that rides for free in the output pass:

    (x-65)@(y-160) = x~@(y~ - 32) + eps
                   = x~@y~ - 32*rowsum(x~)[m] + eps

where x~ = fp8e4m3(x-65) (so the x-side zero-point offset is zero and
no per-column correction is needed) and y~ = fp8e4m3(y-128) (centered
so |v|<=128 and the y-side quant error is halved; its -32 offset folds
into the per-output-row correction corrA[m] = -32*rowsum(x~)[m],
computed on the host from the *quantized* x). The remaining eps is
zero-mean fp8 cross-term noise: measured max rel err 1.18e-2 on the
actual problem seed (threshold 2e-2). All products and partial sums
are integers < 2^24, so fp32 PSUM accumulation is bit-exact and the
host-side error simulation is a faithful predictor of hardware.

Host staging: x (K-major, transposed) and y are quantized to fp8
bytes and staged CONCATENATED per core ([4096, 512+1024]) so a single
DMA delivers a complete contraction k-range for both operands; corrA
([128, 4] fp32, corrA[p, mg] for output row mg*128+p) rides along as
a tiny side input.

Device kernel (identical SPMD program on all 8 cores):
  - K interleaved across SBUF partitions (k = p*32 + j) so each load
    chunk is 128 large contiguous runs (one per partition); HWDGE
    descriptor generation otherwise serializes the load stream.
  - Progressively-sized load chunks alternate between the SP and ACT
    HWDGE rings (per-ring completion latency is ~2.2us, so two rings
    double the early delivery rate); the first chunk is minimal (one
    k-pair) to unlock the stream as soon as possible. corrA loads
    last on the ACT ring (needed only at output time).
  - PE prewarm (~39 throwaway matmuls, gated only on a GpSimd memset
    -- the first engine queue to boot -- so it starts ~1us in)
    releases the HAM clock gate (1.2 -> 2.4 GHz) during the initial
    load window, which is bounded by the first chunk's DMA completion
    latency (~5-6us).
  - Main stream: 128 DoubleRow fp8 matmuls (16 k-pair-tiles x 4 m x
    2 n), each contracting 2 k-tiles (2x128 virtual K) per pass at
    ~238 ns -- 2x the bf16 matmul roofline. The pair dim is an AP dim
    (stride % 16 == 0), no byte interleave needed. The last
    k-pair-tiles run m-major (and the last m-group n-major) so banks
    retire early and correction+scale+store overlaps the remaining
    matmuls.
  - PSUM -> SBUF as a single fused correction+scale tensor_scalar on
    VectorE: out = (psum + corrA[p]) * SCALE. One store DMA per
    128-row group, alternating HWDGE rings so completions overlap. The
    last bank's matmul and pass are split in half (DVE's semaphore
    pickup after a matmul is ~40ns, so the half gated by the very last
    matmul starts almost immediately), and its store is split by ROWS
    (64x2KB descriptors per ring instead of 128x1KB) so the
    kernel-ending chain (last matmul -> copy -> store -> completion)
    is short.
"""

import numpy as np
import ml_dtypes

import concourse.bass as bass  # noqa: F401  (kept for callers/debugging)
import concourse.mybir as mybir
import concourse.tile as tile
from concourse import bacc
from concourse.bass_utils import run_bass_kernel_spmd

X_ZP, Y_ZP = 65.0, 160.0
SCALE = 0.199 * 0.0215
Y_CENTER = 128.0
YOFF = Y_CENTER - Y_ZP  # -32: y-160 = (y-128) - 32

M, K, N = 2048, 4096, 2048
GM, GN = 4, 2  # core grid: 4 M-blocks x 2 N-blocks
MC, NC = M // GM, N // GN  # 512 x 1024 per-core output block
P = 128  # partitions / k-tile size
NB = 512  # psum bank free size (one fp32 bank; matmul cannot cross banks)
# k-pair-tiles (2 k-tiles each) per load DMA. x and y are staged
# concatenated ([k, mc+nnc]) so one DMA delivers a complete k-pair for
# both operands; chunks alternate between the SP and ACT HWDGE rings,
# doubling the completion rate. Small leading chunks start the
# pipeline, big trailing chunks amortize per-DMA completion latency.
DMA_CHUNKS = (1, 1, 2, 2, 3, 3, 2, 2)
JT_TAIL = 8  # trailing k-pair-tiles run m-major so PSUM banks retire early
# Prewarm fills the PE-idle window until the first chunk's DMA
# completion lands (~5us): covers the HAM ramp exactly.
N_WARM = 39


def _emit(tc, zq, mc, corrA, out, dma_chunks=DMA_CHUNKS,
          jt_tail=JT_TAIL, n_warm=N_WARM):
    """Emit the per-core device program.

    zq: [k, mc+nnc] f8 DRAM ([x~ K-major | y~] concatenated),
    corrA: [P, mt] fp32 DRAM, out: [mc, nnc] fp32 DRAM.
    """
    nc = tc.nc
    k, w = zq.shape
    nnc = w - mc
    kt = k // P
    jt = kt // 2  # DoubleRow k-pair tiles
    mt = mc // P
    nt = nnc // NB
    assert 2 * sum(dma_chunks) == kt

    fp32 = mybir.dt.float32
    bf16 = mybir.dt.bfloat16
    f8 = mybir.dt.float8e4

    with (
        tc.tile_pool(name="sb", bufs=1) as sbp,
        tc.tile_pool(name="osb", bufs=mt, space="SBUF") as osbp,
        tc.tile_pool(name="ps", bufs=mt * nt, space="PSUM") as psp,
    ):
        # Everything is persistent (fits in SBUF at this problem size).
        zs = sbp.tile([P, kt, mc + nnc], f8, name="zs")
        xs = zs[:, :, 0:mc]
        ys = zs[:, :, mc:]
        ca = sbp.tile([P, mt], fp32, name="ca")
        wt = sbp.tile([P, P], bf16, name="wt")
        psum = [
            [psp.tile([P, NB], fp32, tag="ps", name=f"ps_{m}_{n}") for n in range(nt)]
            for m in range(mt)
        ]

        # HAM prewarm: the PE sits idle while the first chunks load;
        # throwaway matmuls release the clock gate to 8/8 before the
        # real stream starts. Gate only on a cheap GpSimd memset --
        # GpSimd's queue is the first to boot, so warmup starts ~0.3us
        # earlier than with a VectorE gate.
        nc.gpsimd.memset(wt[:], 0.0)
        for _ in range(n_warm):
            nc.tensor.matmul(psum[0][0][:, :P], wt[:], wt[:], start=True, stop=True)

        # K is interleaved across partitions (k = p*kt + j): each
        # partition's j-range is one big contiguous DRAM run, so a chunk
        # DMA is 128 descriptors (one per partition). The contraction is
        # a permutation of K, identical for x and y, so the matmul sum
        # is unchanged.
        zqr = zq.rearrange("(p j) m -> p j m", j=kt)
        k0 = 0
        rings = (nc.sync, nc.scalar)
        for i, nj in enumerate(dma_chunks):
            nk = 2 * nj
            rings[i % 2].dma_start(zs[:, k0 : k0 + nk, :], zqr[:, k0 : k0 + nk, :])
            k0 += nk
        # corrA is tiny and only needed by the output pass; load it on
        # the ACT ring after the y chunks so it never delays the stream.
        nc.scalar.dma_start(ca[:], corrA)

        def mm(j2, m, n, nsl=slice(0, NB), psl=slice(0, NB)):
            nc.tensor.matmul(
                psum[m][n][:, psl],
                zs[:, 2 * j2 : 2 * j2 + 2, m * P : (m + 1) * P],
                zs[:, 2 * j2 : 2 * j2 + 2, mc + n * NB + nsl.start : mc + n * NB + nsl.stop],
                start=(j2 == 0),
                stop=(j2 == jt - 1),
                perf_mode=mybir.MatmulPerfMode.DoubleRow,
            )

        # k-outer: touch every psum bank each k-pair-tile so the PE
        # stream stays dense while loads race ahead.
        for j2 in range(jt - jt_tail):
            for m in range(mt):
                for n in range(nt):
                    mm(j2, m, n)
        # m-outer tail: bank group m finishes its K accumulation early so
        # its correction+scale+store overlaps the remaining matmuls. The
        # last m-group runs n-major, and the very last matmul is split
        # in half so the kernel-ending chain is short.
        half = NB // 2
        for m in range(mt):
            if m < mt - 1:
                for j2 in range(jt - jt_tail, jt):
                    for n in range(nt):
                        mm(j2, m, n)
            else:
                for n in range(nt):
                    for j2 in range(jt - jt_tail, jt):
                        if n == nt - 1 and j2 == jt - 1:
                            mm(j2, m, n, nsl=slice(0, half), psl=slice(0, half))
                            mm(j2, m, n, nsl=slice(half, NB), psl=slice(half, NB))
                        else:
                            mm(j2, m, n)

        # Fused correction + scale, PSUM->SBUF: out = (psum + corrA[p])
        # * SCALE on VectorE (ACT helps on the last piece). One store
        # per 128-row group, alternating HWDGE rings so completions
        # overlap; the last group's passes and stores are split so the
        # ending chain (last matmul -> copy -> store) is short.
        def opass(dst, src, m, engine=None):
            (engine or nc.vector).tensor_scalar(
                dst, src, ca[:, m : m + 1], SCALE,
                op0=mybir.AluOpType.add, op1=mybir.AluOpType.mult,
            )

        # Ring/queue assignment keeps the kernel-ending chain unblocked:
        # the ACT engine's strict-FIFO queue must not hold any store
        # whose wait (on a DVE pass) would delay the final activation.
        store_rings = [nc.scalar, nc.sync]
        for m in range(mt):
            osb = osbp.tile([P, nnc], fp32, tag="osb", name=f"osb_{m}")
            ring = store_rings[m % 2]
            last_ring = nc.sync
            if m < mt - 1:
                for n in range(nt):
                    opass(osb[:, n * NB : (n + 1) * NB], psum[m][n][:], m)
                ring.dma_start(out[m * P : (m + 1) * P, :], osb[:])
            else:
                # n=0 bank retires 4 k-pair-tiles early (n-major tail):
                # pass + store while the last matmuls still run.
                opass(osb[:, 0:NB], psum[m][0][:], m)
                last_ring.dma_start(out[m * P : (m + 1) * P, 0:NB], osb[:, 0:NB])
                # n=1 bank: both halves on DVE back-to-back. DVE's
                # semaphore pickup is ~40ns vs ACT's ~610ns (measured
                # consistently), so even serialized DVE halves finish
                # ~200ns sooner than a parallel DVE+ACT split: halfA
                # starts at its matmul (one MM before last), halfB
                # follows immediately, ending ~0.9us after the last
                # matmul.
                opass(osb[:, NB : NB + half], psum[m][1][:, 0:half], m)
                opass(osb[:, NB + half :], psum[m][1][:, half:], m)
                # Store the n=1 block split by ROWS, not columns: each
                # half is 64 descriptors of 2KB instead of 128 of 1KB,
                # halving the per-descriptor completion storm that
                # dominates the kernel-ending DMA; the halves ride
                # different rings in parallel.
                # 3-way row split: the SWDGE (gpsimd) ring takes a
                # middle slice so each HWDGE ring's final completion
                # storm shrinks from 64 to 48 descriptors.
                r1, r2 = 48, 96
                nc.scalar.dma_start(
                    out[m * P : m * P + r1, NB:], osb[:r1, NB:]
                )
                nc.gpsimd.dma_start(
                    out[m * P + r1 : m * P + r2, NB:], osb[r1:r2, NB:]
                )
                nc.sync.dma_start(
                    out[m * P + r2 : (m + 1) * P, NB:], osb[r2:, NB:]
                )


def _build_nc(k=K, mc=MC, nnc=NC, **emit_kw):
    nc = bacc.Bacc("TRN2", target_bir_lowering=False, debug=False)
    zq = nc.declare_dram_parameter(
        "zq", [k, mc + nnc], mybir.dt.float8e4, isOutput=False
    )
    corrA = nc.declare_dram_parameter(
        "corrA", [P, mc // P], mybir.dt.float32, isOutput=False
    )
    out = nc.declare_dram_parameter("out", [mc, nnc], mybir.dt.float32, isOutput=True)
    with tile.TileContext(nc) as tc:
        _emit(tc, zq[:], mc, corrA[:], out[:], **emit_kw)
    nc.compile()
    return nc


_CACHE = {}


def _get_nc():
    if "nc" not in _CACHE:
        _CACHE["nc"] = _build_nc()
    return _CACHE["nc"]


def kernel(x, y):
    x = np.asarray(x)
    y = np.asarray(y)
    assert x.shape == (M, K) and y.shape == (K, N)
    f8 = ml_dtypes.float8_e4m3
    # fp8 quantization (values guaranteed 0..255 by the spec): x keeps
    # its true zero-point (no per-column correction needed), y centered.
    xq = (x.astype(np.float32) - X_ZP).astype(f8)  # [M, K]
    yq = (y.astype(np.float32) - Y_CENTER).astype(f8)  # [K, N]
    # Exact rank-1 correction computed from the *quantized* x.
    Rx = xq.astype(np.float32).sum(axis=1, dtype=np.float64)  # [M]
    corrA_full = (YOFF * Rx).astype(np.float32)  # [M]
    xqT = np.ascontiguousarray(xq.T)  # [K, M] K-major

    in_maps = []
    for i in range(GM * GN):
        mi, ni = divmod(i, GN)
        in_maps.append(
            {
                # x (K-major) and y concatenated so one DMA chunk
                # delivers both operands for a k-range
                "zq": np.ascontiguousarray(
                    np.concatenate(
                        [
                            xqT[:, mi * MC : (mi + 1) * MC],
                            yq[:, ni * NC : (ni + 1) * NC],
                        ],
                        axis=1,
                    )
                ),
                # corrA[p, mg] covers output row mg*128 + p of this block
                "corrA": np.ascontiguousarray(
                    corrA_full[mi * MC : (mi + 1) * MC].reshape(MC // P, P).T
                ),
            }
        )

    res = run_bass_kernel_spmd(_get_nc(), in_maps, list(range(GM * GN)))
    _CACHE["last_results"] = res

    out = np.empty((M, N), np.float32)
    for i in range(GM * GN):
        mi, ni = divmod(i, GN)
        out[mi * MC : (mi + 1) * MC, ni * NC : (ni + 1) * NC] = res.results[i]["out"]
    return out
